# revision 13
# baseline (speedup 1.0000x reference)
"""Trainium2 Bass kernel for nn_ConvAttention (sparse_attention).

Reference computes, per batch b and query position i (along L):
    qkv = W1 @ x (1x1 conv);  Q,K,V split
    S[b,i,j] = conv5x5(Q[b,i] + K[b,j]) + b2
    attn     = softmax_j(S)
    out[b,i] = sum_j attn[b,i,j] * V[b,j]

Key algebra (exact): conv is linear, so conv(Q_i+K_j) = conv(Q_i)+conv(K_j);
the Q_i, b2, and conv(b1k) terms are constant along the softmax axis j and
cancel exactly.  attn is therefore independent of i and
    out = sum_j softmax_j(conv5x5(W1k @ x_j)) * (W1v @ x_j + b1v)
with the 1x1 K-projection folded into the conv weights on the host:
    W2eff[o,c,dy,dx] = sum_k W2[o,k,dy,dx] * W1k[k,c].

Sharding: 8 cores = 2 batches x 4 row-quads.  Core m owns batch m//4 and
output rows 4q..4q+3 (q = m%4); it holds input rows 4q-2..4q+5 as four
row-PAIR tiles with partitions = (row parity s, channel c).  This packs the
5x5 conv as dense 128x128 matmuls: contraction (2 rows x 64 c_in), output
(2 out-rows x 64 c_out), 15 matmuls of N=512 per out-row-pair (83% dense vs
the 50% of batch-block-diagonal packing).  All data moves in bf16 (DMA is
~330 GB/s effective; halving bytes halves the load time).

Schedule: input DMAs are chunked in consumption order so the first conv
matmul's semaphore fires just after the 3us PE p-state cliff (matmuls
dispatched later than that run at the full 2.4 GHz).  Score banks are split
(7,7,2)-wide per row-pair so softmax chains (ACT exp -> DVE mul/reduce, all
bf16 SBUF for the DVE fast modes) start while the conv is still running;
the last bank is tiny to shorten the serial tail.  exp-sums and exp*V-sums
ship to the host, which does the final divide, adds b1v, and broadcasts
over l (attn is i-independent).
"""

import numpy as np

B, C, H, W, L = 2, 64, 16, 16, 32
NCORES = 8
QH = 4                      # owned output rows per core
NPAIR = 4                   # input row-pair tiles per core (8 halo rows)
WPAD = W + 4                # zero-padded width
P = 2 * C                   # partitions: (row parity, channel)
WCS = [(0, 7), (7, 7), (14, 2)]   # score-bank column chunks (start, len)

_PLAN = None


def _np_bf16():
    import ml_dtypes
    return ml_dtypes.bfloat16


class _Plan:
    def __init__(self):
        import concourse.bacc as bacc
        import concourse.tile as tile
        from concourse import mybir

        f32 = mybir.dt.float32
        bf16 = mybir.dt.bfloat16
        nc = bacc.Bacc("TRN2", target_bir_lowering=False, debug=False,
                       num_devices=NCORES)

        xa_d = nc.dram_tensor("xa", [P, 2, W, L], bf16, kind="ExternalInput")
        xb_d = nc.dram_tensor("xb", [P, 2, W, L], bf16, kind="ExternalInput")
        wa_d = nc.dram_tensor("wa", [P, 5, P], bf16, kind="ExternalInput")
        wb_d = nc.dram_tensor("wb", [P, 11, P], bf16, kind="ExternalInput")
        o_d = nc.dram_tensor("o", [P, 2, 2, W], bf16, kind="ExternalOutput")

        with tile.TileContext(nc) as tc:
            with (
                tc.tile_pool(name="sb", bufs=1) as sb,
                tc.tile_pool(name="psum", bufs=1, space="PSUM") as psum,
            ):
                xq = sb.tile([P, NPAIR, WPAD, L], bf16, tag="xq")
                wt = sb.tile([P, 16, P], bf16, tag="wt")
                tick = sb.tile([P, 1856], bf16, tag="tick")

                nc.vector.memset(tick[:], 0)
                # Zero the width-pad columns (2 either side of each pair
                # tile); the DMAs below only fill the valid 16-wide region.
                nc.vector.memset(xq[:, :, 0:2, :], 0)
                nc.vector.memset(xq[:, :, 2 + W:, :], 0)

                # Input DMAs in consumption order.  The first conv group
                # needs (wa, xq pair 0); wb (p1, p2, wv) and xb (pairs 2,3)
                # stream in behind.
                nc.sync.dma_start(out=wt[:, 0:5, :], in_=wa_d[:])
                nc.sync.dma_start(out=xq[:, 0:2, 2:2 + W, :], in_=xa_d[:])
                nc.sync.dma_start(out=wt[:, 5:16, :], in_=wb_d[:])
                nc.sync.dma_start(out=xq[:, 2:4, 2:2 + W, :], in_=xb_d[:])

                scores = {(rb, wci): psum.tile([P, n, L], f32,
                                               tag=f"s{rb}{wci}",
                                               name=f"s{rb}{wci}")
                          for rb in range(2)
                          for wci, (ws, n) in enumerate(WCS)}
                vps = [psum.tile([P, W, L], f32, tag=f"vp{t}", name=f"vp{t}")
                       for t in range(2)]
                vsb = [sb.tile([P, W, L], bf16, tag=f"vs{t}", name=f"vs{t}")
                       for t in range(2)]
                osum = [sb.tile([P, 2, W], bf16, tag=f"os{rb}",
                        name=f"os{rb}") for rb in range(2)]

                # PE keep-warm: the p-state clock resets if PE idles for
                # multiple microseconds, and a matmul dispatched within 3us
                # of the reset runs at 1.2 GHz instead of 2.4.  Tiny dummy
                # matmuls paced by a DVE self-copy chain keep PE gaps short
                # through the DMA lead-in, so every real matmul (dispatched
                # after t=3us) runs at full clock.  The dummies write the V2
                # PSUM bank, which is overwritten (start=True) later.
                for k in range(10):
                    src, dst = (0, 928) if k % 2 == 0 else (928, 0)
                    nc.vector.tensor_copy(out=tick[:, dst:dst + 928],
                                          in_=tick[:, src:src + 928])
                    nc.tensor.matmul(vps[1][:, 0:2, :],
                                     lhsT=tick[:, dst:dst + 128],
                                     rhs=tick[:, dst:dst + 64],
                                     start=True, stop=True)

                def conv_group(rb, p):
                    t = rb + p
                    for wci, (ws, n) in enumerate(WCS):
                        for dx in range(5):
                            nc.tensor.matmul(
                                scores[(rb, wci)][:],
                                lhsT=wt[:, 5 * p + dx, :],
                                rhs=xq[:, t, dx + ws:dx + ws + n, :],
                                start=(p == 0 and dx == 0),
                                stop=(p == 2 and dx == 4),
                            )

                def conv_bank_major(rb):
                    # All inputs are resident by the time this runs; ordering
                    # taps bank-by-bank makes each bank finish as early as
                    # possible so its softmax chain overlaps the remaining
                    # conv work.
                    for wci, (ws, n) in enumerate(WCS):
                        for p in range(3):
                            for dx in range(5):
                                nc.tensor.matmul(
                                    scores[(rb, wci)][:],
                                    lhsT=wt[:, 5 * p + dx, :],
                                    rhs=xq[:, rb + p, dx + ws:dx + ws + n, :],
                                    start=(p == 0 and dx == 0),
                                    stop=(p == 2 and dx == 4),
                                )

                def v_proj(rb):
                    # V for out rows (2rb, 2rb+1) = input pair t = rb+1.
                    nc.tensor.matmul(vps[rb][:], lhsT=wt[:, 15, :],
                                     rhs=xq[:, rb + 1, 2:2 + W, :],
                                     start=True, stop=True)
                    nc.scalar.copy(vsb[rb][:], vps[rb][:])

                def chain(rb, wci):
                    # E and E*V share one tile so a single reduce yields both
                    # sums (host does the final divide).
                    ws, n = WCS[wci]
                    ee = sb.tile([P, 2, n, L], bf16, tag=f"e{rb}{wci}",
                                 name=f"e{rb}{wci}")
                    nc.scalar.activation(
                        ee[:, 0], scores[(rb, wci)][:],
                        func=mybir.ActivationFunctionType.Exp)
                    nc.vector.tensor_mul(ee[:, 1], ee[:, 0],
                                         vsb[rb][:, ws:ws + n, :])
                    with nc.allow_low_precision(
                            reason="32-term bf16 sums; rel tol 2e-2"):
                        nc.vector.tensor_reduce(
                            out=osum[rb][:, :, ws:ws + n], in_=ee[:],
                            axis=mybir.AxisListType.X,
                            op=mybir.AluOpType.add)

                conv_group(0, 0)
                conv_group(0, 1)
                v_proj(0)
                conv_group(0, 2)
                v_proj(1)
                for wci in range(3):
                    chain(0, wci)
                nc.sync.dma_start(out=o_d[:, 0], in_=osum[0][:])
                conv_bank_major(1)
                for wci in range(3):
                    chain(1, wci)
                nc.sync.dma_start(out=o_d[:, 1], in_=osum[1][:])

        nc.compile()
        self.nc = nc


def _get_plan():
    global _PLAN
    if _PLAN is None:
        _PLAN = _Plan()
    return _PLAN


def _prep_in_maps(x, W1, W2):
    bf16 = _np_bf16()

    # Fold the K-projection into the conv weights (float64 for accuracy).
    W1k = W1[C:2 * C, :, 0, 0].astype(np.float64)            # [k, c]
    W2eff = np.einsum("okyx,kc->ocyx", W2.astype(np.float64),
                      W1k).astype(np.float32)                # [o, c, 5, 5]
    W1v = W1[2 * C:3 * C, :, 0, 0].astype(np.float32)        # [o, c]

    # Conv lhsT tiles: wt[p*5+dx][(s,ci),(rh,co)] = W2eff[co,ci,2p+s-rh,dx].
    wtiles = np.zeros((15, P, P), np.float32)
    for p in range(3):
        for dx in range(5):
            for s in range(2):
                for rh in range(2):
                    dyi = 2 * p + s - rh
                    if 0 <= dyi <= 4:
                        wtiles[5 * p + dx,
                               64 * s:64 * s + 64,
                               64 * rh:64 * rh + 64] = W2eff[:, :, dyi, dx].T
    # V lhsT: block-diagonal (s,ci)->(s,co) copies of W1v.T.
    wv = np.zeros((P, P), np.float32)
    wv[:C, :C] = W1v.T
    wv[C:, C:] = W1v.T

    wa = np.ascontiguousarray(wtiles[:5].transpose(1, 0, 2)).astype(bf16)
    wb = np.concatenate([wtiles[5:].transpose(1, 0, 2),
                         wv[:, None, :]], axis=1).astype(bf16)

    # x row-pair tiles: [(s,c), t, w, l] = x[bm, c, 4q-2+2t+s, w, l].
    xp = np.zeros((B, C, H + 4, W, L), np.float32)
    xp[:, :, 2:2 + H] = x
    in_maps = []
    for m in range(NCORES):
        bm, q = m // 4, m % 4
        rows = xp[bm, :, 4 * q:4 * q + 8]                    # [c, 8, w, l]
        tiles = rows.reshape(C, NPAIR, 2, W, L).transpose(2, 0, 1, 3, 4)
        tiles = tiles.reshape(P, NPAIR, W, L).astype(bf16)   # [(s,c),t,w,l]
        in_maps.append({
            "xa": np.ascontiguousarray(tiles[:, 0:2]),
            "xb": np.ascontiguousarray(tiles[:, 2:4]),
            "wa": wa, "wb": wb,
        })
    return in_maps


def kernel(x, W1, b1, W2, b2):
    from concourse.bass_utils import run_bass_kernel_spmd

    x = np.asarray(x, dtype=np.float32)
    W1 = np.asarray(W1, dtype=np.float32)
    b1 = np.asarray(b1, dtype=np.float32)
    W2 = np.asarray(W2, dtype=np.float32)

    plan = _get_plan()
    in_maps = _prep_in_maps(x, W1, W2)
    res = run_bass_kernel_spmd(plan.nc, in_maps, core_ids=list(range(NCORES)))

    b1v = b1[2 * C:3 * C].astype(np.float32)
    out = np.empty((B, C, H, W, L), np.float32)
    for m in range(NCORES):
        bm, q = m // 4, m % 4
        o = np.asarray(res.results[m]["o"], dtype=np.float32)
        o = o.reshape(2, C, 2, 2, W)            # [rh, c, rb, E/EV, w]
        val = o[:, :, :, 1] / o[:, :, :, 0]     # [rh, c, rb, w]
        val = val + b1v[None, :, None, None]
        # rows: 4q + 2*rb + rh
        val = val.transpose(1, 2, 0, 3).reshape(C, 4, W)
        out[bm, :, 4 * q:4 * q + 4, :, :] = val[..., None]
    return out


# revision 15
# speedup vs baseline: 1.2234x; 1.2234x over previous
"""Trainium2 Bass kernel for nn_ConvAttention (sparse_attention).

Reference computes, per batch b and query position i (along L):
    qkv = W1 @ x (1x1 conv);  Q,K,V split
    S[b,i,j] = conv5x5(Q[b,i] + K[b,j]) + b2
    attn     = softmax_j(S)
    out[b,i] = sum_j attn[b,i,j] * V[b,j]

Key algebra (exact): conv is linear, so conv(Q_i+K_j) = conv(Q_i)+conv(K_j);
the Q_i, b2, and conv(b1k) terms are constant along the softmax axis j and
cancel exactly.  attn is therefore independent of i and
    out = sum_j softmax_j(conv5x5(W1k @ x_j)) * (W1v @ x_j + b1v)
with the 1x1 K-projection folded into the conv weights on the host:
    W2eff[o,c,dy,dx] = sum_k W2[o,k,dy,dx] * W1k[k,c].

Sharding: 8 cores = 2 batches x 4 row-quads.  Core m owns batch m//4 and
output rows 4q..4q+3 (q = m%4); it holds input rows 4q-2..4q+5 as four
row-PAIR tiles with partitions = (row parity s, channel c).  This packs the
5x5 conv as dense 128x128 matmuls: contraction (2 rows x 64 c_in), output
(2 out-rows x 64 c_out), 15 matmuls of N=512 per out-row-pair (83% dense vs
the 50% of batch-block-diagonal packing).  All data moves in bf16 (DMA is
~330 GB/s effective; halving bytes halves the load time).

Schedule: input DMAs are chunked in consumption order so the first conv
matmul's semaphore fires just after the 3us PE p-state cliff (matmuls
dispatched later than that run at the full 2.4 GHz).  Score banks are split
(7,7,2)-wide per row-pair so softmax chains (ACT exp -> DVE mul/reduce, all
bf16 SBUF for the DVE fast modes) start while the conv is still running;
the last bank is tiny to shorten the serial tail.  exp-sums and exp*V-sums
ship to the host, which does the final divide, adds b1v, and broadcasts
over l (attn is i-independent).
"""

import numpy as np

B, C, H, W, L = 2, 64, 16, 16, 32
NCORES = 8
QH = 4                      # owned output rows per core
NPAIR = 4                   # input row-pair tiles per core (8 halo rows)
WPAD = W + 4                # zero-padded width
P = 2 * C                   # partitions: (row parity, channel)
WCS = [(0, 7), (7, 7), (14, 2)]   # score-bank column chunks (start, len)

_PLAN = None


def _np_bf16():
    import ml_dtypes
    return ml_dtypes.bfloat16


class _Plan:
    def __init__(self):
        import concourse.bacc as bacc
        import concourse.tile as tile
        from concourse import mybir

        f32 = mybir.dt.float32
        bf16 = mybir.dt.bfloat16
        nc = bacc.Bacc("TRN2", target_bir_lowering=False, debug=False,
                       num_devices=NCORES)

        xa_d = nc.dram_tensor("xa", [P, 2, W, L], bf16, kind="ExternalInput")
        xb_d = nc.dram_tensor("xb", [P, 2, W, L], bf16, kind="ExternalInput")
        wa_d = nc.dram_tensor("wa", [P, 5, P], bf16, kind="ExternalInput")
        wb_d = nc.dram_tensor("wb", [P, 11, P], bf16, kind="ExternalInput")
        o_d = nc.dram_tensor("o", [P, 2, 2, W], bf16, kind="ExternalOutput")

        with tile.TileContext(nc) as tc:
            with (
                tc.tile_pool(name="sb", bufs=1) as sb,
                tc.tile_pool(name="psum", bufs=1, space="PSUM") as psum,
            ):
                xq = sb.tile([P, NPAIR, WPAD, L], bf16, tag="xq")
                wt = sb.tile([P, 16, P], bf16, tag="wt")
                wdum = sb.tile([P, 512], bf16, tag="wdum")

                nc.gpsimd.memset(wdum[:], 0)
                # Zero the width-pad columns (2 either side of each pair
                # tile); the DMAs below only fill the valid 16-wide region.
                nc.vector.memset(xq[:, :, 0:2, :], 0)
                nc.vector.memset(xq[:, :, 2 + W:, :], 0)

                # Input DMAs in consumption order.  The first conv group
                # needs (xq pair 0, wa); wb (p1, p2, wv) and xb (pairs 2,3)
                # stream in behind.
                nc.sync.dma_start(out=xq[:, 0:2, 2:2 + W, :], in_=xa_d[:])
                nc.sync.dma_start(out=wt[:, 0:5, :], in_=wa_d[:])
                nc.sync.dma_start(out=wt[:, 5:16, :], in_=wb_d[:])
                nc.sync.dma_start(out=xq[:, 2:4, 2:2 + W, :], in_=xb_d[:])

                scores = {(rb, wci): psum.tile([P, n, L], f32,
                                               tag=f"s{rb}{wci}",
                                               name=f"s{rb}{wci}")
                          for rb in range(2)
                          for wci, (ws, n) in enumerate(WCS)}
                vps = [psum.tile([P, W, L], f32, tag=f"vp{t}", name=f"vp{t}")
                       for t in range(2)]
                vsb = [sb.tile([P, W, L], bf16, tag=f"vs{t}", name=f"vs{t}")
                       for t in range(2)]
                osum = [sb.tile([P, 2, W], bf16, tag=f"os{rb}",
                        name=f"os{rb}") for rb in range(2)]

                # PE keep-warm: the p-state clock resets if PE idles for
                # multiple microseconds, and a matmul dispatched within 3us
                # of the reset runs at half clock.  A ~3.4us accumulation
                # group of dummy matmuls keeps PE busy through the DMA
                # lead-in so the real conv (dispatched after t>4us) runs at
                # the full 2.4 GHz.  The dummies write the V2 PSUM bank,
                # which is reset (start=True) by the real V projection later.
                for k in range(8):
                    nc.tensor.matmul(vps[1][:], lhsT=wdum[:, 0:128],
                                     rhs=wdum[:], start=(k == 0),
                                     stop=(k == 7))

                def conv_group(rb, p):
                    t = rb + p
                    for wci, (ws, n) in enumerate(WCS):
                        for dx in range(5):
                            nc.tensor.matmul(
                                scores[(rb, wci)][:],
                                lhsT=wt[:, 5 * p + dx, :],
                                rhs=xq[:, t, dx + ws:dx + ws + n, :],
                                start=(p == 0 and dx == 0),
                                stop=(p == 2 and dx == 4),
                            )

                def conv_bank_major(rb):
                    # All inputs are resident by the time this runs; ordering
                    # taps bank-by-bank makes each bank finish as early as
                    # possible so its softmax chain overlaps the remaining
                    # conv work.
                    for wci, (ws, n) in enumerate(WCS):
                        for p in range(3):
                            for dx in range(5):
                                nc.tensor.matmul(
                                    scores[(rb, wci)][:],
                                    lhsT=wt[:, 5 * p + dx, :],
                                    rhs=xq[:, rb + p, dx + ws:dx + ws + n, :],
                                    start=(p == 0 and dx == 0),
                                    stop=(p == 2 and dx == 4),
                                )

                def v_proj(rb):
                    # V for out rows (2rb, 2rb+1) = input pair t = rb+1.
                    nc.tensor.matmul(vps[rb][:], lhsT=wt[:, 15, :],
                                     rhs=xq[:, rb + 1, 2:2 + W, :],
                                     start=True, stop=True)
                    nc.scalar.copy(vsb[rb][:], vps[rb][:])

                def chain(rb, wci):
                    # E and E*V share one tile so a single reduce yields both
                    # sums (host does the final divide).
                    ws, n = WCS[wci]
                    ee = sb.tile([P, 2, n, L], bf16, tag=f"e{rb}{wci}",
                                 name=f"e{rb}{wci}")
                    nc.scalar.activation(
                        ee[:, 0], scores[(rb, wci)][:],
                        func=mybir.ActivationFunctionType.Exp)
                    nc.vector.tensor_mul(ee[:, 1], ee[:, 0],
                                         vsb[rb][:, ws:ws + n, :])
                    with nc.allow_low_precision(
                            reason="32-term bf16 sums; rel tol 2e-2"):
                        nc.vector.tensor_reduce(
                            out=osum[rb][:, :, ws:ws + n], in_=ee[:],
                            axis=mybir.AxisListType.X,
                            op=mybir.AluOpType.add)

                conv_group(0, 0)
                conv_group(0, 1)
                v_proj(0)
                conv_group(0, 2)
                v_proj(1)
                for wci in range(3):
                    chain(0, wci)
                nc.sync.dma_start(out=o_d[:, 0], in_=osum[0][:])
                conv_bank_major(1)
                for wci in range(3):
                    chain(1, wci)
                nc.sync.dma_start(out=o_d[:, 1], in_=osum[1][:])

        nc.compile()
        self.nc = nc


def _get_plan():
    global _PLAN
    if _PLAN is None:
        _PLAN = _Plan()
    return _PLAN


def _prep_in_maps(x, W1, W2):
    bf16 = _np_bf16()

    # Fold the K-projection into the conv weights (float64 for accuracy).
    W1k = W1[C:2 * C, :, 0, 0].astype(np.float64)            # [k, c]
    W2eff = np.einsum("okyx,kc->ocyx", W2.astype(np.float64),
                      W1k).astype(np.float32)                # [o, c, 5, 5]
    W1v = W1[2 * C:3 * C, :, 0, 0].astype(np.float32)        # [o, c]

    # Conv lhsT tiles: wt[p*5+dx][(s,ci),(rh,co)] = W2eff[co,ci,2p+s-rh,dx].
    wtiles = np.zeros((15, P, P), np.float32)
    for p in range(3):
        for dx in range(5):
            for s in range(2):
                for rh in range(2):
                    dyi = 2 * p + s - rh
                    if 0 <= dyi <= 4:
                        wtiles[5 * p + dx,
                               64 * s:64 * s + 64,
                               64 * rh:64 * rh + 64] = W2eff[:, :, dyi, dx].T
    # V lhsT: block-diagonal (s,ci)->(s,co) copies of W1v.T.
    wv = np.zeros((P, P), np.float32)
    wv[:C, :C] = W1v.T
    wv[C:, C:] = W1v.T

    wa = np.ascontiguousarray(wtiles[:5].transpose(1, 0, 2)).astype(bf16)
    wb = np.concatenate([wtiles[5:].transpose(1, 0, 2),
                         wv[:, None, :]], axis=1).astype(bf16)

    # x row-pair tiles: [(s,c), t, w, l] = x[bm, c, 4q-2+2t+s, w, l].
    xp = np.zeros((B, C, H + 4, W, L), np.float32)
    xp[:, :, 2:2 + H] = x
    in_maps = []
    for m in range(NCORES):
        bm, q = m // 4, m % 4
        rows = xp[bm, :, 4 * q:4 * q + 8]                    # [c, 8, w, l]
        tiles = rows.reshape(C, NPAIR, 2, W, L).transpose(2, 0, 1, 3, 4)
        tiles = tiles.reshape(P, NPAIR, W, L).astype(bf16)   # [(s,c),t,w,l]
        in_maps.append({
            "xa": np.ascontiguousarray(tiles[:, 0:2]),
            "xb": np.ascontiguousarray(tiles[:, 2:4]),
            "wa": wa, "wb": wb,
        })
    return in_maps


def kernel(x, W1, b1, W2, b2):
    from concourse.bass_utils import run_bass_kernel_spmd

    x = np.asarray(x, dtype=np.float32)
    W1 = np.asarray(W1, dtype=np.float32)
    b1 = np.asarray(b1, dtype=np.float32)
    W2 = np.asarray(W2, dtype=np.float32)

    plan = _get_plan()
    in_maps = _prep_in_maps(x, W1, W2)
    res = run_bass_kernel_spmd(plan.nc, in_maps, core_ids=list(range(NCORES)))

    b1v = b1[2 * C:3 * C].astype(np.float32)
    out = np.empty((B, C, H, W, L), np.float32)
    for m in range(NCORES):
        bm, q = m // 4, m % 4
        o = np.asarray(res.results[m]["o"], dtype=np.float32)
        o = o.reshape(2, C, 2, 2, W)            # [rh, c, rb, E/EV, w]
        val = o[:, :, :, 1] / o[:, :, :, 0]     # [rh, c, rb, w]
        val = val + b1v[None, :, None, None]
        # rows: 4q + 2*rb + rh
        val = val.transpose(1, 2, 0, 3).reshape(C, 4, W)
        out[bm, :, 4 * q:4 * q + 4, :, :] = val[..., None]
    return out


# revision 19
# speedup vs baseline: 1.2848x; 1.0502x over previous
"""Trainium2 Bass kernel for nn_ConvAttention (sparse_attention).

Reference computes, per batch b and query position i (along L):
    qkv = W1 @ x (1x1 conv);  Q,K,V split
    S[b,i,j] = conv5x5(Q[b,i] + K[b,j]) + b2
    attn     = softmax_j(S)
    out[b,i] = sum_j attn[b,i,j] * V[b,j]

Key algebra (exact): conv is linear, so conv(Q_i+K_j) = conv(Q_i)+conv(K_j);
the Q_i, b2, and conv(b1k) terms are constant along the softmax axis j and
cancel exactly.  attn is therefore independent of i and
    out = sum_j softmax_j(conv5x5(W1k @ x_j)) * (W1v @ x_j + b1v)
with the 1x1 K-projection folded into the conv weights on the host:
    W2eff[o,c,dy,dx] = sum_k W2[o,k,dy,dx] * W1k[k,c].

Sharding: 8 cores = 2 batches x 4 row-quads.  Core m owns batch m//4 and
output rows 4q..4q+3 (q = m%4); it holds input rows 4q-2..4q+5 as four
row-PAIR tiles with partitions = (row parity s, channel c).  This packs the
5x5 conv as dense 128x128 matmuls: contraction (2 rows x 64 c_in), output
(2 out-rows x 64 c_out), 15 matmuls of N=512 per out-row-pair (83% dense vs
the 50% of batch-block-diagonal packing).  All data moves in bf16 (DMA is
~330 GB/s effective; halving bytes halves the load time).

Schedule: input DMAs are chunked in consumption order so the first conv
matmul's semaphore fires just after the 3us PE p-state cliff (matmuls
dispatched later than that run at the full 2.4 GHz).  Score banks are split
(7,7,2)-wide per row-pair so softmax chains (ACT exp -> DVE mul/reduce, all
bf16 SBUF for the DVE fast modes) start while the conv is still running;
the last bank is tiny to shorten the serial tail.  exp-sums and exp*V-sums
ship to the host, which does the final divide, adds b1v, and broadcasts
over l (attn is i-independent).
"""

import numpy as np

B, C, H, W, L = 2, 64, 16, 16, 32
NCORES = 8
QH = 4                      # owned output rows per core
NPAIR = 4                   # input row-pair tiles per core (8 halo rows)
WPAD = W + 4                # zero-padded width
P = 2 * C                   # partitions: (row parity, channel)
# Score-bank column chunks (start, len) per row-pair.  rb0's banks finish
# mid-conv (chains fully hidden), so two wide banks minimize op overhead;
# rb1's banks finish near the end, so a train of small banks keeps each
# softmax chain short and the final one tiny.
WCS0 = [(0, 8), (8, 8)]
WCS1 = [(0, 7), (7, 3), (10, 3), (13, 3)]

_PLAN = None


def _np_bf16():
    import ml_dtypes
    return ml_dtypes.bfloat16


class _Plan:
    def __init__(self):
        import concourse.bacc as bacc
        import concourse.tile as tile
        from concourse import mybir

        f32 = mybir.dt.float32
        bf16 = mybir.dt.bfloat16
        nc = bacc.Bacc("TRN2", target_bir_lowering=False, debug=False,
                       num_devices=NCORES)

        xa_d = nc.dram_tensor("xa", [P, 2, W, L], bf16, kind="ExternalInput")
        xb_d = nc.dram_tensor("xb", [P, 2, W, L], bf16, kind="ExternalInput")
        wa_d = nc.dram_tensor("wa", [P, 5, P], bf16, kind="ExternalInput")
        wb_d = nc.dram_tensor("wb", [P, 11, P], bf16, kind="ExternalInput")
        o_d = nc.dram_tensor("o", [P, 2, 2, W], bf16, kind="ExternalOutput")

        with tile.TileContext(nc) as tc:
            with (
                tc.tile_pool(name="sb", bufs=1) as sb,
                tc.tile_pool(name="psum", bufs=1, space="PSUM") as psum,
            ):
                xq = sb.tile([P, NPAIR, WPAD, L], bf16, tag="xq")
                wt = sb.tile([P, 16, P], bf16, tag="wt")
                wdum = sb.tile([P, 128], bf16, tag="wdum")

                nc.gpsimd.memset(wdum[:], 0)
                # Zero the width-pad columns (2 either side of each pair
                # tile); the DMAs below only fill the valid 16-wide region.
                nc.vector.memset(xq[:, :, 0:2, :], 0)
                nc.vector.memset(xq[:, :, 2 + W:, :], 0)

                # Input DMAs in consumption order.  The first conv group
                # needs (xq pair 0, wa); wb (p1, p2, wv) and xb (pairs 2,3)
                # stream in behind.
                nc.sync.dma_start(out=xq[:, 0:2, 2:2 + W, :], in_=xa_d[:])
                nc.sync.dma_start(out=wt[:, 0:5, :], in_=wa_d[:])
                nc.sync.dma_start(out=wt[:, 5:16, :], in_=wb_d[:])
                nc.sync.dma_start(out=xq[:, 2:4, 2:2 + W, :], in_=xb_d[:])

                scores = {(rb, wci): psum.tile([P, n, L], f32,
                                               tag=f"s{rb}{wci}",
                                               name=f"s{rb}{wci}")
                          for rb, wcs in ((0, WCS0), (1, WCS1))
                          for wci, (ws, n) in enumerate(wcs)}
                vps = [psum.tile([P, W, L], f32, tag=f"vp{t}", name=f"vp{t}")
                       for t in range(2)]
                vsb = [sb.tile([P, W, L], bf16, tag=f"vs{t}", name=f"vs{t}")
                       for t in range(2)]
                osum = [sb.tile([P, 2, W], bf16, tag=f"os{rb}",
                        name=f"os{rb}") for rb in range(2)]

                # PE keep-warm: the p-state clock resets if PE idles for
                # multiple microseconds, and a matmul dispatched within 3us
                # of the reset runs at half clock.  A ~3.2us accumulation
                # group of dummy matmuls keeps PE busy through the DMA
                # lead-in so the real conv (dispatched after t>4us) runs at
                # the full 2.4 GHz.  The dummies write the V2 PSUM bank,
                # which is reset (start=True) by the real V projection later.
                NWARM = 30
                for k in range(NWARM):
                    nc.tensor.matmul(vps[1][:, 0:4, :], lhsT=wdum[:],
                                     rhs=wdum[:], start=(k == 0),
                                     stop=(k == NWARM - 1))

                def conv_group(rb, p, wcs):
                    t = rb + p
                    for wci, (ws, n) in enumerate(wcs):
                        for dx in range(5):
                            nc.tensor.matmul(
                                scores[(rb, wci)][:],
                                lhsT=wt[:, 5 * p + dx, :],
                                rhs=xq[:, t, dx + ws:dx + ws + n, :],
                                start=(p == 0 and dx == 0),
                                stop=(p == 2 and dx == 4),
                            )

                def conv_bank(rb, wci, ws, n):
                    # All inputs resident: order taps bank-by-bank so each
                    # bank finishes as early as possible and its softmax
                    # chain overlaps the remaining conv work.
                    for p in range(3):
                        for dx in range(5):
                            nc.tensor.matmul(
                                scores[(rb, wci)][:],
                                lhsT=wt[:, 5 * p + dx, :],
                                rhs=xq[:, rb + p, dx + ws:dx + ws + n, :],
                                start=(p == 0 and dx == 0),
                                stop=(p == 2 and dx == 4),
                            )

                def chain(rb, wci, ws, n, mul_engine):
                    # E and E*V share one tile so a single reduce yields both
                    # sums (host does the final divide).  rb0's muls go to
                    # the otherwise-idle gpsimd to keep DVE (the tail
                    # bottleneck) free for the reduces.
                    ee = sb.tile([P, 2, n, L], bf16, tag=f"e{rb}{wci}",
                                 name=f"e{rb}{wci}")
                    nc.scalar.activation(
                        ee[:, 0], scores[(rb, wci)][:],
                        func=mybir.ActivationFunctionType.Exp)
                    mul_engine.tensor_mul(ee[:, 1], ee[:, 0],
                                          vsb[rb][:, ws:ws + n, :])
                    with nc.allow_low_precision(
                            reason="32-term bf16 sums; rel tol 2e-2"):
                        nc.vector.tensor_reduce(
                            out=osum[rb][:, :, ws:ws + n], in_=ee[:],
                            axis=mybir.AxisListType.X,
                            op=mybir.AluOpType.add)

                conv_group(0, 0, WCS0)
                conv_group(0, 1, WCS0)
                # V projections: V for out rows (2rb, 2rb+1) = pair rb+1.
                nc.tensor.matmul(vps[0][:], lhsT=wt[:, 15, :],
                                 rhs=xq[:, 1, 2:2 + W, :],
                                 start=True, stop=True)
                nc.scalar.copy(vsb[0][:], vps[0][:])
                nc.tensor.matmul(vps[1][:], lhsT=wt[:, 15, :],
                                 rhs=xq[:, 2, 2:2 + W, :],
                                 start=True, stop=True)
                conv_group(0, 2, WCS0)
                for wci, (ws, n) in enumerate(WCS0):
                    chain(0, wci, ws, n, nc.gpsimd)
                # V2's SBUF copy sits on ACT after rb0's exps so it doesn't
                # delay them; rb1's first mul needs it ~1.5us later.
                nc.scalar.copy(vsb[1][:], vps[1][:])
                nc.sync.dma_start(out=o_d[:, 0], in_=osum[0][:])
                for wci, (ws, n) in enumerate(WCS1):
                    conv_bank(1, wci, ws, n)
                    chain(1, wci, ws, n, nc.vector)
                nc.sync.dma_start(out=o_d[:, 1], in_=osum[1][:])

        nc.compile()
        self.nc = nc


def _get_plan():
    global _PLAN
    if _PLAN is None:
        _PLAN = _Plan()
    return _PLAN


def _prep_in_maps(x, W1, W2):
    bf16 = _np_bf16()

    # Fold the K-projection into the conv weights (float64 for accuracy).
    W1k = W1[C:2 * C, :, 0, 0].astype(np.float64)            # [k, c]
    W2eff = np.einsum("okyx,kc->ocyx", W2.astype(np.float64),
                      W1k).astype(np.float32)                # [o, c, 5, 5]
    W1v = W1[2 * C:3 * C, :, 0, 0].astype(np.float32)        # [o, c]

    # Conv lhsT tiles: wt[p*5+dx][(s,ci),(rh,co)] = W2eff[co,ci,2p+s-rh,dx].
    wtiles = np.zeros((15, P, P), np.float32)
    for p in range(3):
        for dx in range(5):
            for s in range(2):
                for rh in range(2):
                    dyi = 2 * p + s - rh
                    if 0 <= dyi <= 4:
                        wtiles[5 * p + dx,
                               64 * s:64 * s + 64,
                               64 * rh:64 * rh + 64] = W2eff[:, :, dyi, dx].T
    # V lhsT: block-diagonal (s,ci)->(s,co) copies of W1v.T.
    wv = np.zeros((P, P), np.float32)
    wv[:C, :C] = W1v.T
    wv[C:, C:] = W1v.T

    wa = np.ascontiguousarray(wtiles[:5].transpose(1, 0, 2)).astype(bf16)
    wb = np.concatenate([wtiles[5:].transpose(1, 0, 2),
                         wv[:, None, :]], axis=1).astype(bf16)

    # x row-pair tiles: [(s,c), t, w, l] = x[bm, c, 4q-2+2t+s, w, l].
    xp = np.zeros((B, C, H + 4, W, L), np.float32)
    xp[:, :, 2:2 + H] = x
    in_maps = []
    for m in range(NCORES):
        bm, q = m // 4, m % 4
        rows = xp[bm, :, 4 * q:4 * q + 8]                    # [c, 8, w, l]
        tiles = rows.reshape(C, NPAIR, 2, W, L).transpose(2, 0, 1, 3, 4)
        tiles = tiles.reshape(P, NPAIR, W, L).astype(bf16)   # [(s,c),t,w,l]
        in_maps.append({
            "xa": np.ascontiguousarray(tiles[:, 0:2]),
            "xb": np.ascontiguousarray(tiles[:, 2:4]),
            "wa": wa, "wb": wb,
        })
    return in_maps


def kernel(x, W1, b1, W2, b2):
    from concourse.bass_utils import run_bass_kernel_spmd

    x = np.asarray(x, dtype=np.float32)
    W1 = np.asarray(W1, dtype=np.float32)
    b1 = np.asarray(b1, dtype=np.float32)
    W2 = np.asarray(W2, dtype=np.float32)

    plan = _get_plan()
    in_maps = _prep_in_maps(x, W1, W2)
    res = run_bass_kernel_spmd(plan.nc, in_maps, core_ids=list(range(NCORES)))

    b1v = b1[2 * C:3 * C].astype(np.float32)
    out = np.empty((B, C, H, W, L), np.float32)
    for m in range(NCORES):
        bm, q = m // 4, m % 4
        o = np.asarray(res.results[m]["o"], dtype=np.float32)
        o = o.reshape(2, C, 2, 2, W)            # [rh, c, rb, E/EV, w]
        val = o[:, :, :, 1] / o[:, :, :, 0]     # [rh, c, rb, w]
        val = val + b1v[None, :, None, None]
        # rows: 4q + 2*rb + rh
        val = val.transpose(1, 2, 0, 3).reshape(C, 4, W)
        out[bm, :, 4 * q:4 * q + 4, :, :] = val[..., None]
    return out
